# revision 12
# baseline (speedup 1.0000x reference)
"""Distributed causal multi-head attention on one TRN2 chip (8 NeuronCores).

Problem: B=2, S=2048, D=1024, H=16, DH=64 (f32), causal softmax attention with
QKV + output projections.

Sharding (SPMD, one Bass graph for all 8 cores):
  core i -> batch b = i // 4, head group g = i % 4 (4 of 16 heads).
Each core projects Q/K/V for its 4 heads over the full sequence of its batch
and runs causal attention.  The per-head attention outputs z (bf16) are
AllGathered within each batch's 4-core group, one 512-row band at a time, so
each core holds z for ALL 16 heads of its band and computes a 256-column
slice of the output projection.  Core (b, g) returns out[b, :, 256g:256g+256];
the host assembles the full output by pure concatenation.  The per-band
AllGather overlaps with the next band's attention compute (software pipeline,
output projection runs one band behind).

Layout choices (all on-chip transposes avoided):
  - activations are fed pre-transposed from host: x^T [D, S] (layout prep only)
  - Q/K produced in [e, s] layout (heads pair-packed on partitions), fp32r
    matmuls (full PE rate at N>=256, f32 storage)
  - scores computed transposed [k, s_q] so softmax needs no cross-partition
    reduce: exp without max-subtraction (scores are O(+-3), fp32-safe), the
    denominator comes for free from a ones-column appended to V
  - V produced in [s, e] layout, interleaved per head with a ones column
  - attention matmuls in bf16 (PSUM accumulation in f32)
  - normalization by 1/den is applied via a rank-1 broadcast matmul
  - each PSUM accumulation group gets its own bank (start=True clears the
    whole bank, so groups must never share one)
"""

import sys

for _p in ("/opt/trn_rl_repo", "/opt/pypackages"):
    if _p not in sys.path:
        sys.path.insert(0, _p)

from contextlib import ExitStack

import numpy as np

import concourse.bass as bass
import concourse.mybir as mybir
import concourse.tile as tile
from concourse import bacc
from concourse.bass_utils import run_bass_kernel_spmd

B, S, D, H, DH = 2, 2048, 1024, 16, 64
G = 4                       # heads per core
NCORES = 8
SCALE = float(np.sqrt(DH))
TQ = 512                    # query tile (free dim)
NQT = S // TQ               # 4
KC = 128                    # key chunk (partition dim)
NKC = S // KC               # 16
DC = 128                    # contraction d-chunk
NDC = D // DC               # 8
EG = G * DH                 # 256: packed head dim per group
VW = DH + 1                 # 65: head slot width in v_aug (ones column)
DS = D // 4                 # 256: output D-column slice per core
NEG = -1.0e9

F32 = mybir.dt.float32
F32R = mybir.dt.float32r
BF16 = mybir.dt.bfloat16

EXP = mybir.ActivationFunctionType.Exp

GROUPS = [[0, 1, 2, 3], [4, 5, 6, 7]]

_CACHE = {}


def _build() -> bass.Bass:
    nc = bacc.Bacc("TRN2", num_devices=NCORES, target_bir_lowering=False)

    xq = nc.declare_dram_parameter("xq", [D, S], BF16, isOutput=False)
    xk = nc.declare_dram_parameter("xk", [D, S], BF16, isOutput=False)
    xv = nc.declare_dram_parameter("xv", [D, S], BF16, isOutput=False)
    wq = nc.declare_dram_parameter("wq", [NDC, DC, EG], BF16, isOutput=False)
    wk = nc.declare_dram_parameter("wk", [NDC, DC, EG], BF16, isOutput=False)
    wv = nc.declare_dram_parameter("wv", [NDC, DC, EG], BF16, isOutput=False)
    wo = nc.declare_dram_parameter("wo", [NDC, DC, DS], BF16, isOutput=False)
    mask = nc.declare_dram_parameter("mask", [KC, G * TQ], F32, isOutput=False)
    out_ext = nc.declare_dram_parameter("out", [S, DS], F32, isOutput=True)

    with ExitStack() as ctx:
        tc = ctx.enter_context(tile.TileContext(nc))
        const = ctx.enter_context(tc.tile_pool(name="const", bufs=1))
        dram = ctx.enter_context(tc.tile_pool(name="dram", bufs=1, space="DRAM"))
        xpool = ctx.enter_context(tc.tile_pool(name="x", bufs=6))
        spool = ctx.enter_context(tc.tile_pool(name="s", bufs=3))
        epool = ctx.enter_context(tc.tile_pool(name="e", bufs=4))
        rpool = ctx.enter_context(tc.tile_pool(name="r", bufs=2))
        zgpool = ctx.enter_context(tc.tile_pool(name="zg", bufs=2))
        opool = ctx.enter_context(tc.tile_pool(name="o", bufs=3))
        psum = ctx.enter_context(tc.tile_pool(name="psum", bufs=8, space="PSUM"))

        # ------- constants -------
        wq_sb = const.tile([DC, NDC * EG], BF16, name="wq_sb")
        wk_sb = const.tile([DC, NDC * EG], BF16, name="wk_sb")
        wv_sb = const.tile([DC, NDC * EG], BF16, name="wv_sb")
        for c in range(NDC):
            nc.sync.dma_start(wq_sb[:, c * EG:(c + 1) * EG], wq[c])
            nc.sync.dma_start(wk_sb[:, c * EG:(c + 1) * EG], wk[c])
            nc.sync.dma_start(wv_sb[:, c * EG:(c + 1) * EG], wv[c])
        wo_sb = const.tile([DC, NDC * DS], BF16, name="wo_sb")
        for c in range(NDC):
            nc.sync.dma_start(wo_sb[:, c * DS:(c + 1) * DS], wo[c])
        mask_sb = const.tile([KC, G * TQ], F32, name="mask_sb")
        nc.sync.dma_start(mask_sb[:], mask[:, :])
        ones_sb = const.tile([1, DH], BF16, name="ones_sb")
        nc.vector.memset(ones_sb[:], 1.0)
        # v_aug: per k-chunk, per head: 64 value cols + 1 ones col (ones come
        # from this memset; the value copies below leave them untouched)
        vaug = const.tile([KC, NKC * G * VW], BF16, name="vaug")
        nc.gpsimd.memset(vaug[:], 1.0)

        q_sb = [const.tile([2 * DH, S], BF16, name=f"q_sb{p}") for p in range(2)]
        k_sb = [const.tile([2 * DH, S], BF16, name=f"k_sb{p}") for p in range(2)]
        z_sb = [const.tile([2 * DH, S], BF16, name=f"z_sb{p}") for p in range(2)]

        # ------- projection helpers (called per band from the main loop) ----
        # q/k -> [e (pair-packed), s] (q pre-scaled by 1/sqrt(DH));
        # v -> [s, e] interleaved into v_aug.
        def proj_band(t):
            for xin, wsb, dst, is_q in (
                (xq, wq_sb, q_sb, True),
                (xk, wk_sb, k_sb, False),
            ):
                pq = [
                    psum.tile([128, TQ], F32, tag="ps", name=f"pq{p}")
                    for p in range(2)
                ]
                for c in range(NDC):
                    x_t = xpool.tile([DC, TQ], BF16, name="x_t", tag="x")
                    nc.sync.dma_start(
                        x_t[:], xin[c * DC:(c + 1) * DC, t * TQ:(t + 1) * TQ]
                    )
                    for p in range(2):
                        nc.tensor.matmul(
                            pq[p][:],
                            wsb[:, c * EG + p * 128: c * EG + (p + 1) * 128],
                            x_t[:],
                            start=(c == 0),
                            stop=(c == NDC - 1),
                        )
                for p in range(2):
                    if is_q:
                        nc.vector.tensor_scalar_mul(
                            dst[p][:, t * TQ:(t + 1) * TQ], pq[p][:], 1.0 / SCALE
                        )
                    else:
                        nc.vector.tensor_copy(
                            dst[p][:, t * TQ:(t + 1) * TQ], pq[p][:]
                        )
            # v: keep the 8 x-chunks resident, run each s-sub's accumulation
            # in its own bank, sub-outer so only ~2 banks are live at a time
            vx = [
                xpool.tile([DC, TQ], BF16, name=f"vx{c}", tag=f"vx{c}")
                for c in range(NDC)
            ]
            for c in range(NDC):
                nc.sync.dma_start(
                    vx[c][:], xv[c * DC:(c + 1) * DC, t * TQ:(t + 1) * TQ]
                )
            for sub in range(4):
                pv = psum.tile([128, EG], F32, tag="ps", name="pv")
                for c in range(NDC):
                    nc.tensor.matmul(
                        pv[:],
                        vx[c][:, sub * KC:(sub + 1) * KC],
                        wv_sb[:, c * EG:(c + 1) * EG],
                        start=(c == 0),
                        stop=(c == NDC - 1),
                    )
                kci = t * 4 + sub
                for h in range(G):
                    nc.vector.tensor_copy(
                        vaug[:, kci * G * VW + h * VW: kci * G * VW + h * VW + DH],
                        pv[:, h * DH:(h + 1) * DH],
                    )

        # ------- per-band DRAM staging for the z AllGather -------
        zb = [dram.tile([EG, TQ], BF16, name=f"zb{t}") for t in range(NQT)]
        zg = [
            dram.tile([G * EG, TQ], BF16, name=f"zg{t}") for t in range(NQT)
        ]

        def normalize(t, h, pz, recip):
            # z / den via rank-1 broadcast of 1/den on the PE
            p_i, off = h // 2, (h % 2) * DH
            pb = psum.tile([128, TQ], F32, tag="ps", name="pb")
            nc.tensor.matmul(
                pb[0:DH, :], ones_sb[:], recip[:], start=True, stop=True
            )
            bc = rpool.tile([DH, TQ], F32, name="bc", tag="bc")
            nc.vector.tensor_copy(bc[:], pb[0:DH, :])
            nc.vector.tensor_mul(
                z_sb[p_i][off:off + DH, t * TQ:(t + 1) * TQ], pz[0:DH, :], bc[:]
            )

        def attention_band(t):
            nkc = 4 * t + 4        # causal: only k-chunks <= diagonal
            pzs = []
            for h in range(G):
                p_i, off = h // 2, (h % 2) * DH
                qt, kt = q_sb[p_i], k_sb[p_i]
                pz = psum.tile([128, TQ], F32, tag="ps", name=f"pz{h}")
                for kci in range(nkc):
                    psc = psum.tile([128, TQ], F32, tag="ps", name="psc")
                    nc.tensor.matmul(
                        psc[:],
                        kt[off:off + DH, kci * KC:(kci + 1) * KC],
                        qt[off:off + DH, t * TQ:(t + 1) * TQ],
                        start=True,
                        stop=True,
                    )
                    e_t = epool.tile([KC, TQ], BF16, name="e_t", tag="e")
                    dc = kci - 4 * t
                    if dc >= 0:        # diagonal chunk: additive causal mask
                        m_t = spool.tile([KC, TQ], F32, name="m_t", tag="m")
                        nc.vector.tensor_add(
                            m_t[:], psc[:], mask_sb[:, dc * TQ:(dc + 1) * TQ]
                        )
                        nc.scalar.activation(e_t[:], m_t[:], EXP)
                    else:
                        nc.scalar.activation(e_t[:], psc[:], EXP)
                    nc.tensor.matmul(
                        pz[0:VW, :],
                        vaug[:, kci * G * VW + h * VW: kci * G * VW + (h + 1) * VW],
                        e_t[:],
                        start=(kci == 0),
                        stop=(kci == nkc - 1),
                    )
                # 1/den on DVE right away (overlaps the next head's matmuls)
                recip = rpool.tile([1, TQ], BF16, name="recip", tag="recip")
                with nc.allow_low_precision(reason="softmax denom recip, bf16"):
                    nc.vector.reciprocal(recip[:], pz[DH:DH + 1, :])
                pzs.append((h, pz, recip))
                # one-head lag: normalize head h-1 now that its recip is
                # certainly done -- the PE broadcast never waits on the DVE
                if len(pzs) == 2:
                    normalize(t, *pzs.pop(0))
            normalize(t, *pzs.pop(0))
            # stage this band's z and AllGather it across the 4-core group
            for p in range(2):
                nc.sync.dma_start(
                    zb[t][p * 128:(p + 1) * 128, :],
                    z_sb[p][:, t * TQ:(t + 1) * TQ],
                )
            nc.gpsimd.collective_compute(
                "AllGather",
                mybir.AluOpType.bypass,
                replica_groups=GROUPS,
                ins=[zb[t].opt()],
                outs=[zg[t].opt()],
            )

        def oproj_band(t):
            # out[512t:512(t+1), :] = z_all(band t)^T @ W_O[:, cols of this core]
            zg_sb = zgpool.tile([128, NDC * TQ], BF16, name="zg_sb", tag="zg")
            for c in range(NDC):
                nc.sync.dma_start(
                    zg_sb[:, c * TQ:(c + 1) * TQ], zg[t][c * DC:(c + 1) * DC, :]
                )
            for qs in range(4):
                po = psum.tile([128, TQ], F32, tag="ps", name="po")
                for c in range(NDC):
                    nc.tensor.matmul(
                        po[:, 0:DS],
                        zg_sb[:, c * TQ + qs * KC: c * TQ + (qs + 1) * KC],
                        wo_sb[:, c * DS:(c + 1) * DS],
                        start=(c == 0),
                        stop=(c == NDC - 1),
                    )
                o_sb = opool.tile([KC, DS], F32, name="o_sb", tag="o")
                nc.vector.tensor_copy(o_sb[:], po[:, 0:DS])
                nc.sync.dma_start(
                    out_ext[t * TQ + qs * KC: t * TQ + (qs + 1) * KC, :], o_sb[:]
                )

        # software pipeline: projections feed the same band's attention,
        # band t's AllGather overlaps band t+1's work, and the output
        # projection runs one band behind.
        for t in range(NQT):
            proj_band(t)
            attention_band(t)
            if t >= 1:
                oproj_band(t - 1)
        oproj_band(NQT - 1)

    nc.compile()
    return nc


def _get_graph() -> bass.Bass:
    if "nc" not in _CACHE:
        _CACHE["nc"] = _build()
    return _CACHE["nc"]


def _make_mask() -> np.ndarray:
    m = np.empty((KC, G * TQ), np.float32)
    x = np.arange(KC)[:, None]
    y = np.arange(TQ)[None, :]
    for dc in range(G):
        m[:, dc * TQ:(dc + 1) * TQ] = np.where(dc * KC + x <= y, 0.0, NEG)
    return m


def _make_in_maps(inputs: dict) -> list[dict]:
    import ml_dtypes

    bf16 = ml_dtypes.bfloat16
    qx = np.asarray(inputs["query_input"], np.float32).astype(bf16)
    kx = np.asarray(inputs["key_input"], np.float32).astype(bf16)
    vx = np.asarray(inputs["value_input"], np.float32).astype(bf16)
    WQ = np.asarray(inputs["W_Q"], np.float32).astype(bf16)
    WK = np.asarray(inputs["W_K"], np.float32).astype(bf16)
    WV = np.asarray(inputs["W_V"], np.float32).astype(bf16)
    WO = np.asarray(inputs["W_O"], np.float32).astype(bf16)

    mask = _make_mask()
    xT = {
        (nm, b): np.ascontiguousarray(arr[b].T)
        for nm, arr in (("xq", qx), ("xk", kx), ("xv", vx))
        for b in range(B)
    }
    WO_flat = WO.reshape(H * DH, D)   # e' = h*64 + e, h-major (AllGather order)
    wmaps = []
    for g in range(G):
        hs = slice(g * G, (g + 1) * G)

        def prep(w):
            return np.ascontiguousarray(
                w[hs].transpose(1, 0, 2).reshape(D, EG).reshape(NDC, DC, EG)
            )

        wmaps.append(
            {
                "wq": prep(WQ),
                "wk": prep(WK),
                "wv": prep(WV),
                "wo": np.ascontiguousarray(
                    WO_flat[:, g * DS:(g + 1) * DS].reshape(NDC, DC, DS)
                ),
            }
        )

    in_maps = []
    for core in range(NCORES):
        b, g = core // G, core % G
        m = {
            "xq": xT[("xq", b)],
            "xk": xT[("xk", b)],
            "xv": xT[("xv", b)],
            "mask": mask,
        }
        m.update(wmaps[g])
        in_maps.append(m)
    return in_maps


def _assemble(results: list[dict]) -> np.ndarray:
    out = np.empty((B, S, D), np.float32)
    for core in range(NCORES):
        b, g = core // G, core % G
        out[b, :, g * DS:(g + 1) * DS] = results[core]["out"]
    return out


def run(inputs: dict, trace: bool = False):
    """Run on hardware; returns (output, BassKernelResults)."""
    nc = _get_graph()
    res = run_bass_kernel_spmd(
        nc, _make_in_maps(inputs), core_ids=list(range(NCORES)), trace=trace
    )
    return _assemble(res.results), res


def kernel(**inputs) -> np.ndarray:
    out, _ = run(inputs)
    return out


# revision 13
# speedup vs baseline: 1.0708x; 1.0708x over previous
"""Distributed causal multi-head attention on one TRN2 chip (8 NeuronCores).

Problem: B=2, S=2048, D=1024, H=16, DH=64 (f32), causal softmax attention with
QKV + output projections.

Sharding (SPMD, one Bass graph for all 8 cores):
  core i -> batch b = i // 4, head group g = i % 4 (4 of 16 heads).
Each core projects Q/K/V for its 4 heads over the full sequence of its batch
and runs causal attention.  The per-head attention outputs z (bf16) are
AllGathered within each batch's 4-core group, one 512-row band at a time, so
each core holds z for ALL 16 heads of its band and computes a 256-column
slice of the output projection.  Core (b, g) returns out[b, :, 256g:256g+256];
the host assembles the full output by pure concatenation.  The per-band
AllGather overlaps with the next band's attention compute (software pipeline,
output projection runs one band behind).

Layout choices (all on-chip transposes avoided):
  - activations are fed pre-transposed from host: x^T [D, S] (layout prep only)
  - Q/K produced in [e, s] layout (heads pair-packed on partitions), fp32r
    matmuls (full PE rate at N>=256, f32 storage)
  - scores computed transposed [k, s_q] so softmax needs no cross-partition
    reduce: exp without max-subtraction (scores are O(+-3), fp32-safe), the
    denominator comes for free from a ones-column appended to V
  - V produced in [s, e] layout, interleaved per head with a ones column
  - attention matmuls in bf16 (PSUM accumulation in f32)
  - normalization by 1/den is applied via a rank-1 broadcast matmul
  - each PSUM accumulation group gets its own bank (start=True clears the
    whole bank, so groups must never share one)
"""

import sys

for _p in ("/opt/trn_rl_repo", "/opt/pypackages"):
    if _p not in sys.path:
        sys.path.insert(0, _p)

from contextlib import ExitStack

import numpy as np

import concourse.bass as bass
import concourse.mybir as mybir
import concourse.tile as tile
from concourse import bacc
from concourse.bass_utils import run_bass_kernel_spmd

B, S, D, H, DH = 2, 2048, 1024, 16, 64
G = 4                       # heads per core
NCORES = 8
SCALE = float(np.sqrt(DH))
TQ = 512                    # query tile (free dim)
NQT = S // TQ               # 4
KC = 128                    # key chunk (partition dim)
NKC = S // KC               # 16
DC = 128                    # contraction d-chunk
NDC = D // DC               # 8
EG = G * DH                 # 256: packed head dim per group
VW = DH + 1                 # 65: head slot width in v_aug (ones column)
DS = D // 4                 # 256: output D-column slice per core
NEG = -1.0e9

F32 = mybir.dt.float32
F32R = mybir.dt.float32r
BF16 = mybir.dt.bfloat16

EXP = mybir.ActivationFunctionType.Exp

GROUPS = [[0, 1, 2, 3], [4, 5, 6, 7]]

_CACHE = {}


def _build() -> bass.Bass:
    nc = bacc.Bacc("TRN2", num_devices=NCORES, target_bir_lowering=False)

    xq = nc.declare_dram_parameter("xq", [D, S], BF16, isOutput=False)
    xk = nc.declare_dram_parameter("xk", [D, S], BF16, isOutput=False)
    xv = nc.declare_dram_parameter("xv", [D, S], BF16, isOutput=False)
    wq = nc.declare_dram_parameter("wq", [NDC, DC, EG], BF16, isOutput=False)
    wk = nc.declare_dram_parameter("wk", [NDC, DC, EG], BF16, isOutput=False)
    wv = nc.declare_dram_parameter("wv", [NDC, DC, EG], BF16, isOutput=False)
    wo = nc.declare_dram_parameter("wo", [NDC, DC, DS], BF16, isOutput=False)
    mask = nc.declare_dram_parameter("mask", [KC, G * TQ], F32, isOutput=False)
    out_ext = nc.declare_dram_parameter("out", [S, DS], F32, isOutput=True)

    with ExitStack() as ctx:
        tc = ctx.enter_context(tile.TileContext(nc))
        const = ctx.enter_context(tc.tile_pool(name="const", bufs=1))
        dram = ctx.enter_context(tc.tile_pool(name="dram", bufs=1, space="DRAM"))
        xpool = ctx.enter_context(tc.tile_pool(name="x", bufs=6))
        spool = ctx.enter_context(tc.tile_pool(name="s", bufs=3))
        epool = ctx.enter_context(tc.tile_pool(name="e", bufs=4))
        rpool = ctx.enter_context(tc.tile_pool(name="r", bufs=2))
        zgpool = ctx.enter_context(tc.tile_pool(name="zg", bufs=2))
        opool = ctx.enter_context(tc.tile_pool(name="o", bufs=3))
        psum = ctx.enter_context(tc.tile_pool(name="psum", bufs=8, space="PSUM"))

        # ------- constants -------
        wq_sb = const.tile([DC, NDC * EG], BF16, name="wq_sb")
        wk_sb = const.tile([DC, NDC * EG], BF16, name="wk_sb")
        wv_sb = const.tile([DC, NDC * EG], BF16, name="wv_sb")
        for c in range(NDC):
            nc.sync.dma_start(wq_sb[:, c * EG:(c + 1) * EG], wq[c])
            nc.sync.dma_start(wk_sb[:, c * EG:(c + 1) * EG], wk[c])
            nc.sync.dma_start(wv_sb[:, c * EG:(c + 1) * EG], wv[c])
        wo_sb = const.tile([DC, NDC * DS], BF16, name="wo_sb")
        for c in range(NDC):
            nc.sync.dma_start(wo_sb[:, c * DS:(c + 1) * DS], wo[c])
        mask_sb = const.tile([KC, G * TQ], F32, name="mask_sb")
        nc.sync.dma_start(mask_sb[:], mask[:, :])
        ones_sb = const.tile([1, DH], BF16, name="ones_sb")
        nc.vector.memset(ones_sb[:], 1.0)
        # v_aug: per k-chunk, per head: 64 value cols + 1 ones col (ones come
        # from this memset; the value copies below leave them untouched)
        vaug = const.tile([KC, NKC * G * VW], BF16, name="vaug")
        nc.gpsimd.memset(vaug[:], 1.0)

        q_sb = [const.tile([2 * DH, S], BF16, name=f"q_sb{p}") for p in range(2)]
        k_sb = [const.tile([2 * DH, S], BF16, name=f"k_sb{p}") for p in range(2)]
        z_sb = [const.tile([2 * DH, S], BF16, name=f"z_sb{p}") for p in range(2)]

        # ------- projection helpers (called per band from the main loop) ----
        # q/k -> [e (pair-packed), s] (q pre-scaled by 1/sqrt(DH));
        # v -> [s, e] interleaved into v_aug.
        def proj_band(t):
            for xin, wsb, dst, is_q in (
                (xq, wq_sb, q_sb, True),
                (xk, wk_sb, k_sb, False),
            ):
                pq = [
                    psum.tile([128, TQ], F32, tag="ps", name=f"pq{p}")
                    for p in range(2)
                ]
                for c in range(NDC):
                    x_t = xpool.tile([DC, TQ], BF16, name="x_t", tag="x")
                    nc.sync.dma_start(
                        x_t[:], xin[c * DC:(c + 1) * DC, t * TQ:(t + 1) * TQ]
                    )
                    for p in range(2):
                        nc.tensor.matmul(
                            pq[p][:],
                            wsb[:, c * EG + p * 128: c * EG + (p + 1) * 128],
                            x_t[:],
                            start=(c == 0),
                            stop=(c == NDC - 1),
                        )
                for p in range(2):
                    if is_q:
                        nc.vector.tensor_scalar_mul(
                            dst[p][:, t * TQ:(t + 1) * TQ], pq[p][:], 1.0 / SCALE
                        )
                    else:
                        nc.vector.tensor_copy(
                            dst[p][:, t * TQ:(t + 1) * TQ], pq[p][:]
                        )
            # v: keep the 8 x-chunks resident, run each s-sub's accumulation
            # in its own bank, sub-outer so only ~2 banks are live at a time
            vx = [
                xpool.tile([DC, TQ], BF16, name=f"vx{c}", tag=f"vx{c}", bufs=2)
                for c in range(NDC)
            ]
            for c in range(NDC):
                nc.sync.dma_start(
                    vx[c][:], xv[c * DC:(c + 1) * DC, t * TQ:(t + 1) * TQ]
                )
            for sub in range(4):
                pv = psum.tile([128, EG], F32, tag="ps", name="pv")
                for c in range(NDC):
                    nc.tensor.matmul(
                        pv[:],
                        vx[c][:, sub * KC:(sub + 1) * KC],
                        wv_sb[:, c * EG:(c + 1) * EG],
                        start=(c == 0),
                        stop=(c == NDC - 1),
                    )
                kci = t * 4 + sub
                for h in range(G):
                    nc.vector.tensor_copy(
                        vaug[:, kci * G * VW + h * VW: kci * G * VW + h * VW + DH],
                        pv[:, h * DH:(h + 1) * DH],
                    )

        # ------- per-band DRAM staging for the z AllGather -------
        zb = [dram.tile([EG, TQ], BF16, name=f"zb{t}") for t in range(NQT)]
        zg = [
            dram.tile([G * EG, TQ], BF16, name=f"zg{t}") for t in range(NQT)
        ]

        def normalize(t, h, pz, recip):
            # z / den via rank-1 broadcast of 1/den on the PE
            p_i, off = h // 2, (h % 2) * DH
            pb = psum.tile([128, TQ], F32, tag="ps", name="pb")
            nc.tensor.matmul(
                pb[0:DH, :], ones_sb[:], recip[:], start=True, stop=True
            )
            bc = rpool.tile([DH, TQ], F32, name="bc", tag="bc")
            nc.vector.tensor_copy(bc[:], pb[0:DH, :])
            nc.vector.tensor_mul(
                z_sb[p_i][off:off + DH, t * TQ:(t + 1) * TQ], pz[0:DH, :], bc[:]
            )

        def attention_band(t):
            nkc = 4 * t + 4        # causal: only k-chunks <= diagonal
            pzs = []
            for h in range(G):
                p_i, off = h // 2, (h % 2) * DH
                qt, kt = q_sb[p_i], k_sb[p_i]
                pz = psum.tile([128, TQ], F32, tag="ps", name=f"pz{h}")
                for kci in range(nkc):
                    psc = psum.tile([128, TQ], F32, tag="ps", name="psc")
                    nc.tensor.matmul(
                        psc[:],
                        kt[off:off + DH, kci * KC:(kci + 1) * KC],
                        qt[off:off + DH, t * TQ:(t + 1) * TQ],
                        start=True,
                        stop=True,
                    )
                    e_t = epool.tile([KC, TQ], BF16, name="e_t", tag="e")
                    dc = kci - 4 * t
                    if dc >= 0:        # diagonal chunk: additive causal mask
                        m_t = spool.tile([KC, TQ], F32, name="m_t", tag="m")
                        nc.vector.tensor_add(
                            m_t[:], psc[:], mask_sb[:, dc * TQ:(dc + 1) * TQ]
                        )
                        nc.scalar.activation(e_t[:], m_t[:], EXP)
                    else:
                        nc.scalar.activation(e_t[:], psc[:], EXP)
                    nc.tensor.matmul(
                        pz[0:VW, :],
                        vaug[:, kci * G * VW + h * VW: kci * G * VW + (h + 1) * VW],
                        e_t[:],
                        start=(kci == 0),
                        stop=(kci == nkc - 1),
                    )
                # 1/den on DVE right away (overlaps the next head's matmuls)
                recip = rpool.tile([1, TQ], BF16, name="recip", tag="recip")
                with nc.allow_low_precision(reason="softmax denom recip, bf16"):
                    nc.vector.reciprocal(recip[:], pz[DH:DH + 1, :])
                pzs.append((h, pz, recip))
                # one-head lag: normalize head h-1 now that its recip is
                # certainly done -- the PE broadcast never waits on the DVE
                if len(pzs) == 2:
                    normalize(t, *pzs.pop(0))
            normalize(t, *pzs.pop(0))
            # stage this band's z and AllGather it across the 4-core group
            for p in range(2):
                nc.sync.dma_start(
                    zb[t][p * 128:(p + 1) * 128, :],
                    z_sb[p][:, t * TQ:(t + 1) * TQ],
                )
            nc.gpsimd.collective_compute(
                "AllGather",
                mybir.AluOpType.bypass,
                replica_groups=GROUPS,
                ins=[zb[t].opt()],
                outs=[zg[t].opt()],
            )

        def oproj_band(t):
            # out[512t:512(t+1), :] = z_all(band t)^T @ W_O[:, cols of this core]
            zg_sb = zgpool.tile([128, NDC * TQ], BF16, name="zg_sb", tag="zg")
            for c in range(NDC):
                nc.sync.dma_start(
                    zg_sb[:, c * TQ:(c + 1) * TQ], zg[t][c * DC:(c + 1) * DC, :]
                )
            for qs in range(4):
                po = psum.tile([128, TQ], F32, tag="ps", name="po")
                for c in range(NDC):
                    nc.tensor.matmul(
                        po[:, 0:DS],
                        zg_sb[:, c * TQ + qs * KC: c * TQ + (qs + 1) * KC],
                        wo_sb[:, c * DS:(c + 1) * DS],
                        start=(c == 0),
                        stop=(c == NDC - 1),
                    )
                o_sb = opool.tile([KC, DS], F32, name="o_sb", tag="o")
                nc.vector.tensor_copy(o_sb[:], po[:, 0:DS])
                nc.sync.dma_start(
                    out_ext[t * TQ + qs * KC: t * TQ + (qs + 1) * KC, :], o_sb[:]
                )

        # projections first (DMA-paced), then the attention bands; band t's
        # AllGather overlaps band t+1's attention and the output projection
        # runs one band behind.
        for t in range(NQT):
            proj_band(t)
        for t in range(NQT):
            attention_band(t)
            if t >= 1:
                oproj_band(t - 1)
        oproj_band(NQT - 1)

    nc.compile()
    return nc


def _get_graph() -> bass.Bass:
    if "nc" not in _CACHE:
        _CACHE["nc"] = _build()
    return _CACHE["nc"]


def _make_mask() -> np.ndarray:
    m = np.empty((KC, G * TQ), np.float32)
    x = np.arange(KC)[:, None]
    y = np.arange(TQ)[None, :]
    for dc in range(G):
        m[:, dc * TQ:(dc + 1) * TQ] = np.where(dc * KC + x <= y, 0.0, NEG)
    return m


def _make_in_maps(inputs: dict) -> list[dict]:
    import ml_dtypes

    bf16 = ml_dtypes.bfloat16
    qx = np.asarray(inputs["query_input"], np.float32).astype(bf16)
    kx = np.asarray(inputs["key_input"], np.float32).astype(bf16)
    vx = np.asarray(inputs["value_input"], np.float32).astype(bf16)
    WQ = np.asarray(inputs["W_Q"], np.float32).astype(bf16)
    WK = np.asarray(inputs["W_K"], np.float32).astype(bf16)
    WV = np.asarray(inputs["W_V"], np.float32).astype(bf16)
    WO = np.asarray(inputs["W_O"], np.float32).astype(bf16)

    mask = _make_mask()
    xT = {
        (nm, b): np.ascontiguousarray(arr[b].T)
        for nm, arr in (("xq", qx), ("xk", kx), ("xv", vx))
        for b in range(B)
    }
    WO_flat = WO.reshape(H * DH, D)   # e' = h*64 + e, h-major (AllGather order)
    wmaps = []
    for g in range(G):
        hs = slice(g * G, (g + 1) * G)

        def prep(w):
            return np.ascontiguousarray(
                w[hs].transpose(1, 0, 2).reshape(D, EG).reshape(NDC, DC, EG)
            )

        wmaps.append(
            {
                "wq": prep(WQ),
                "wk": prep(WK),
                "wv": prep(WV),
                "wo": np.ascontiguousarray(
                    WO_flat[:, g * DS:(g + 1) * DS].reshape(NDC, DC, DS)
                ),
            }
        )

    in_maps = []
    for core in range(NCORES):
        b, g = core // G, core % G
        m = {
            "xq": xT[("xq", b)],
            "xk": xT[("xk", b)],
            "xv": xT[("xv", b)],
            "mask": mask,
        }
        m.update(wmaps[g])
        in_maps.append(m)
    return in_maps


def _assemble(results: list[dict]) -> np.ndarray:
    out = np.empty((B, S, D), np.float32)
    for core in range(NCORES):
        b, g = core // G, core % G
        out[b, :, g * DS:(g + 1) * DS] = results[core]["out"]
    return out


def run(inputs: dict, trace: bool = False):
    """Run on hardware; returns (output, BassKernelResults)."""
    nc = _get_graph()
    res = run_bass_kernel_spmd(
        nc, _make_in_maps(inputs), core_ids=list(range(NCORES)), trace=trace
    )
    return _assemble(res.results), res


def kernel(**inputs) -> np.ndarray:
    out, _ = run(inputs)
    return out
